# revision 21
# baseline (speedup 1.0000x reference)
"""Block-sparse position-wise FFN on Trainium2 (Bass/Tile), 8-core data-parallel.

Strategy:
  - Shard tokens (B*S = 36928) evenly across 8 cores: 4616 tokens/core.
    The FFN is pointwise over tokens and both (masked) weight matrices fit
    in SBUF, so data-parallel needs no collectives.
  - Host prep: apply the 8x8 block masks to W1/W2, pre-transpose and
    pre-block everything into the exact SBUF images (so every DMA is
    contiguous 4-6KB descriptors), and cast to bf16 (tolerance is 2e-2;
    bf16 end-to-end lands ~5e-3). The PE does zero transposes.
  - Per core, fused loop over 512-token chunks (PSUM-bank sized):
      hT[m] = gelu(W1mT[:,m].T @ xT + b1[m])     (bf16 matmuls, ACT gelu+bias)
      out[t,:] = hT.T @ W2mT + b2                (natural-layout output)
  - Weights stream in as 8 slab-DMAs per matrix across two DMA queues so
    fc1 starts ~10us in; warm-up matmuls during the DMA wait lift the PE
    HAM clock-gate before real work arrives; output DMAs are split in half
    across three queues so the kernel tail drains fast.
"""

import sys
import types

import numpy as np
import ml_dtypes

# concourse's axon trace path imports antenv.axon_hooks, which this image
# lacks; install a no-op shim so an env-requested trace degrades gracefully
# instead of raising ImportError.
try:
    import antenv.axon_hooks  # noqa: F401
except ImportError:
    import antenv

    _hooks = types.ModuleType("antenv.axon_hooks")
    _hooks._hook = None
    _hooks.set_axon_ntff_profile_hook = (
        lambda h: setattr(_hooks, "_hook", h))
    _hooks.get_axon_ntff_profile_hook = lambda: _hooks._hook
    sys.modules["antenv.axon_hooks"] = _hooks
    antenv.axon_hooks = _hooks

import concourse.bass as bass
import concourse.bacc as bacc
import concourse.mybir as mybir
from concourse import tile
from concourse.bass_utils import run_bass_kernel_spmd

B, S, DIM, FF, BLK = 64, 577, 768, 3072, 8
NCORES = 8
TOK = B * S                # 36928
T = TOK // NCORES          # 4616 tokens per core
P = 128
KD = DIM // P              # 6 k-tiles for fc1
KF = FF // P               # 24 f-tiles
F32 = mybir.dt.float32
BF16 = mybir.dt.bfloat16
BF = ml_dtypes.bfloat16
GELU = mybir.ActivationFunctionType.Gelu
_ACT_FUNC = GELU  # simtest.py overrides with Identity (CoreSim lacks Gelu)

NSLAB = 8
W1SLAB = KF * DIM // NSLAB   # 2304 cols of the [P, KF*DIM] w1 image
W2SLAB = KF * DIM // NSLAB   # 2304 cols of the [P, KF*DIM] w2 image


def _chunks(total):
    """512-token chunks (one PSUM bank of fp32 per fc1 matmul output)."""
    out, pos = [], 0
    while pos < total:
        w = min(512, total - pos)
        out.append((pos, w))
        pos += w
    return out


def _token_tiles(w):
    tiles, off = [], 0
    while off < w:
        p = min(P, w - off)
        tiles.append((off, p))
        off += p
    return tiles


def _body(tc, xb_d, w1_d, b1_d, w2_d, b2_d, o_d, t_tokens):
    nc = tc.nc
    with (
        tc.tile_pool(name="const", bufs=1) as constp,
        tc.tile_pool(name="wpool", bufs=1) as wp,
        tc.tile_pool(name="xt", bufs=3) as xtp,
        tc.tile_pool(name="ht", bufs=50) as htp,
        tc.tile_pool(name="onat", bufs=4) as onatp,
        tc.tile_pool(name="ps1", bufs=4, space=bass.MemorySpace.PSUM) as ps1p,
        tc.tile_pool(name="ps2", bufs=2, space=bass.MemorySpace.PSUM) as ps2p,
    ):
        b1_s = constp.tile([P, KF], F32)
        nc.scalar.dma_start(out=b1_s[:], in_=b1_d)
        b2_s = constp.tile([P, DIM], F32)
        nc.scalar.dma_start(out=b2_s[:], in_=b2_d)

        # Warm-ups during the weight-DMA wait: absorb the ~1.3us Gelu table
        # load on ACT, and keep the PE busy until real matmuls arrive so the
        # HAM clock-gate sits at 8/8 (else the first ~3.4us run at half rate).
        # memset (not DMA) sources the operands so nothing blocks them.
        warm = constp.tile([P, 512], BF16)
        nc.vector.memset(warm[:, :], 0.0)
        warm2 = constp.tile([P, 8], BF16)
        nc.scalar.activation(warm2[:, :], warm[:, 0:8], _ACT_FUNC, bias=0.0)
        psw = ps1p.tile([P, 512], F32, tag="ps1")
        for _ in range(20):
            nc.tensor.matmul(psw[:, :], warm[:, 0:P], warm[:, :],
                             start=True, stop=True)

        # Weight SBUF images, streamed in 8 contiguous slabs each, spread
        # over two DMA queues. w1 image is m-major: col m*768 + k*128 + c
        # holds W1mT[128k+c_row...]; fc1's m-chain for m only needs slab
        # m//3, so matmuls start after ~590KB instead of all of W1.
        chunks = _chunks(t_tokens)

        def load_x(c0, cw):
            """One contiguous DMA for a whole pre-blocked token chunk."""
            xt = xtp.tile([P, KD * cw], BF16, tag="xt", name="xt")
            nc.gpsimd.dma_start(
                out=xt[:, :], in_=xb_d[:, KD * c0:KD * (c0 + cw)]
            )
            return xt

        xt_cur = load_x(*chunks[0])
        # prefetch chunk 1 too: its fc1 interleaves with chunk 0's below
        xt_nxt = load_x(*chunks[1]) if len(chunks) > 1 else None

        w1_s = wp.tile([P, KF * DIM], BF16, tag="w1", name="w1")
        w2_s = wp.tile([P, KF * DIM], BF16, tag="w2", name="w2")
        NS = 6
        SW = KF * DIM // NS           # 3072 cols -> 6KB descriptors,
        # first slab split in half so fc1's m=0 chain starts ~4us sooner
        w1_bounds = [0, SW // 2, SW] + [(s + 1) * SW for s in range(1, NS)]
        w1_q = [nc.sync, nc.scalar, nc.gpsimd, nc.sync, nc.scalar,
                nc.gpsimd, nc.sync]
        for i, (a, b) in enumerate(zip(w1_bounds, w1_bounds[1:])):
            w1_q[i].dma_start(out=w1_s[:, a:b], in_=w1_d[:, a:b])
        for s in range(NS):
            q = nc.sync if s % 2 == 1 else nc.scalar
            q.dma_start(
                out=w2_s[:, s * SW:(s + 1) * SW],
                in_=w2_d[:, s * SW:(s + 1) * SW],
            )

        out_q = [nc.sync, nc.scalar]
        out_n = 0

        def fc1_chain(xt, cw, m):
            ps1 = ps1p.tile([P, cw], F32, tag="ps1")
            for k in range(KD):
                nc.tensor.matmul(
                    ps1[:, :],
                    w1_s[:, m * DIM + k * P:m * DIM + (k + 1) * P],
                    xt[:, k * cw:(k + 1) * cw],
                    start=(k == 0), stop=(k == KD - 1),
                )
            ht = htp.tile([P, cw], BF16, tag="ht")
            nc.scalar.activation(
                ht[:, :], ps1[:, :], _ACT_FUNC, bias=b1_s[:, m:m + 1]
            )
            return ht

        def fc2_chunk(c0, cw, hts):
            nonlocal out_n
            for (toff, tp) in _token_tiles(cw):
                ps2 = ps2p.tile([P, DIM], F32, tag="ps2")
                for k in range(KF):
                    last = (k == KF - 1)
                    for off, wdt in ((0, 512), (512, DIM - 512)):
                        nc.tensor.matmul(
                            ps2[0:tp, off:off + wdt],
                            hts[k][:, toff:toff + tp],
                            w2_s[:, k * DIM + off:k * DIM + off + wdt],
                            start=(k == 0), stop=last,
                        )
                on = onatp.tile([P, DIM], BF16, tag="on")
                nc.vector.tensor_tensor(
                    out=on[0:tp, :], in0=ps2[0:tp, :], in1=b2_s[0:tp, :],
                    op=mybir.AluOpType.add,
                )
                # split the store across two DMA queues so the last store
                # isn't a single ~9us single-engine transfer
                for off, wdt in ((0, 384), (384, 384)):
                    out_q[out_n % 2].dma_start(
                        out=o_d[c0 + toff:c0 + toff + tp, off:off + wdt],
                        in_=on[0:tp, off:off + wdt],
                    )
                    out_n += 1

        # Prologue: interleave chunk 0 and chunk 1 fc1 m-chains so the PE
        # has 2x the compute per W1 slab while the cold DMA rings are still
        # delivering weights (else fc1 outruns the slab stream and stalls).
        if xt_nxt is not None:
            hts0, hts1 = [], []
            for m in range(KF):
                hts0.append(fc1_chain(xt_cur, chunks[0][1], m))
                hts1.append(fc1_chain(xt_nxt, chunks[1][1], m))
            fc2_chunk(chunks[0][0], chunks[0][1], hts0)
            xt_cur = load_x(*chunks[2]) if len(chunks) > 2 else None
            fc2_chunk(chunks[1][0], chunks[1][1], hts1)
            start_ci = 2
        else:
            hts0 = [fc1_chain(xt_cur, chunks[0][1], m) for m in range(KF)]
            fc2_chunk(chunks[0][0], chunks[0][1], hts0)
            start_ci = 1

        for ci in range(start_ci, len(chunks)):
            c0, cw = chunks[ci]
            hts = [fc1_chain(xt_cur, cw, m) for m in range(KF)]
            xt_next = (load_x(*chunks[ci + 1])
                       if ci + 1 < len(chunks) else None)
            fc2_chunk(c0, cw, hts)
            xt_cur = xt_next


def build_program(t_tokens=T):
    nc = bacc.Bacc("TRN2", target_bir_lowering=False, debug=False,
                   num_devices=NCORES)
    xb_d = nc.dram_tensor("xb", [P, KD * t_tokens], BF16,
                          kind="ExternalInput").ap()
    w1_d = nc.dram_tensor("w1b", [P, KF * DIM], BF16,
                          kind="ExternalInput").ap()
    b1_d = nc.dram_tensor("b1", [P, KF], F32, kind="ExternalInput").ap()
    w2_d = nc.dram_tensor("w2b", [P, KF * DIM], BF16,
                          kind="ExternalInput").ap()
    b2_d = nc.dram_tensor("b2", [P, DIM], F32, kind="ExternalInput").ap()
    o_d = nc.dram_tensor("out", [t_tokens, DIM], BF16,
                         kind="ExternalOutput").ap()
    with tile.TileContext(nc) as tc:
        _body(tc, xb_d, w1_d, b1_d, w2_d, b2_d, o_d, t_tokens)
    nc.compile()
    return nc


def host_prep(x, W1, b1, W2, b2, mask1, mask2):
    x2 = np.asarray(x, dtype=np.float32).reshape(TOK, DIM)
    m1 = np.repeat(np.repeat(np.asarray(mask1, dtype=bool), BLK, 0), BLK, 1)
    m2 = np.repeat(np.repeat(np.asarray(mask2, dtype=bool), BLK, 0), BLK, 1)
    w1t = (np.asarray(W1, np.float32) * m1.astype(np.float32)).T  # [DIM, FF]
    w2t = (np.asarray(W2, np.float32) * m2.astype(np.float32)).T  # [FF, DIM]
    # m-major blocked image: w1b[p, m*768 + k*128 + c] = w1t[k*128+p, m*128+c]
    w1b = np.ascontiguousarray(
        w1t.reshape(KD, P, KF, P).transpose(1, 2, 0, 3).reshape(P, KF * DIM)
    ).astype(BF)
    # w2b[p, m*768 + d] = w2t[m*128+p, d]
    w2b = np.ascontiguousarray(
        w2t.reshape(KF, P, DIM).transpose(1, 0, 2).reshape(P, KF * DIM)
    ).astype(BF)
    b1h = np.ascontiguousarray(
        np.asarray(b1, np.float32).reshape(KF, P).T)              # [P, KF]
    b2h = np.ascontiguousarray(
        np.broadcast_to(np.asarray(b2, np.float32)[None, :], (P, DIM)))
    chunks = _chunks(T)
    xbs = []
    for c in range(NCORES):
        xt_c = x2[c * T:(c + 1) * T].T                            # [DIM, T]
        xb_c = np.concatenate(
            [xt_c[:, c0:c0 + cw].reshape(KD, P, cw).transpose(1, 0, 2)
             .reshape(P, KD * cw) for (c0, cw) in chunks], axis=1)
        xbs.append(np.ascontiguousarray(xb_c).astype(BF))
    return xbs, w1b, b1h, w2b, b2h


_PROGRAM = None


def _get_program():
    global _PROGRAM
    if _PROGRAM is None:
        _PROGRAM = build_program(T)
    return _PROGRAM


def kernel(x, W1, b1, W2, b2, mask1, mask2, **run_kwargs):
    xbs, w1b, b1h, w2b, b2h = host_prep(x, W1, b1, W2, b2, mask1, mask2)
    nc = _get_program()
    in_maps = [
        {"xb": xbs[c], "w1b": w1b, "b1": b1h, "w2b": w2b, "b2": b2h}
        for c in range(NCORES)
    ]
    res = run_bass_kernel_spmd(nc, in_maps, list(range(NCORES)), **run_kwargs)
    out = np.concatenate([res.results[c]["out"] for c in range(NCORES)], axis=0)
    out = out.astype(np.float32).reshape(B, S, DIM)
    if run_kwargs:
        kernel.last_results = res
    return out


# revision 22
# speedup vs baseline: 1.0041x; 1.0041x over previous
"""Block-sparse position-wise FFN on Trainium2 (Bass/Tile), 8-core data-parallel.

Strategy:
  - Shard tokens (B*S = 36928) evenly across 8 cores: 4616 tokens/core.
    The FFN is pointwise over tokens and both (masked) weight matrices fit
    in SBUF, so data-parallel needs no collectives.
  - Host prep: apply the 8x8 block masks to W1/W2, pre-transpose and
    pre-block everything into the exact SBUF images (so every DMA is
    contiguous 4-6KB descriptors), and cast to bf16 (tolerance is 2e-2;
    bf16 end-to-end lands ~5e-3). The PE does zero transposes.
  - Per core, fused loop over 512-token chunks (PSUM-bank sized):
      hT[m] = gelu(W1mT[:,m].T @ xT + b1[m])     (bf16 matmuls, ACT gelu+bias)
      out[t,:] = hT.T @ W2mT + b2                (natural-layout output)
  - Weights stream in as 8 slab-DMAs per matrix across two DMA queues so
    fc1 starts ~10us in; warm-up matmuls during the DMA wait lift the PE
    HAM clock-gate before real work arrives; output DMAs are split in half
    across three queues so the kernel tail drains fast.
"""

import sys
import types

import numpy as np
import ml_dtypes

# concourse's axon trace path imports antenv.axon_hooks, which this image
# lacks; install a no-op shim so an env-requested trace degrades gracefully
# instead of raising ImportError.
try:
    import antenv.axon_hooks  # noqa: F401
except ImportError:
    import antenv

    _hooks = types.ModuleType("antenv.axon_hooks")
    _hooks._hook = None
    _hooks.set_axon_ntff_profile_hook = (
        lambda h: setattr(_hooks, "_hook", h))
    _hooks.get_axon_ntff_profile_hook = lambda: _hooks._hook
    sys.modules["antenv.axon_hooks"] = _hooks
    antenv.axon_hooks = _hooks

import concourse.bass as bass
import concourse.bacc as bacc
import concourse.mybir as mybir
from concourse import tile
from concourse.bass_utils import run_bass_kernel_spmd

B, S, DIM, FF, BLK = 64, 577, 768, 3072, 8
NCORES = 8
TOK = B * S                # 36928
T = TOK // NCORES          # 4616 tokens per core
P = 128
KD = DIM // P              # 6 k-tiles for fc1
KF = FF // P               # 24 f-tiles
F32 = mybir.dt.float32
BF16 = mybir.dt.bfloat16
BF = ml_dtypes.bfloat16
GELU = mybir.ActivationFunctionType.Gelu
_ACT_FUNC = GELU  # simtest.py overrides with Identity (CoreSim lacks Gelu)

NSLAB = 8
W1SLAB = KF * DIM // NSLAB   # 2304 cols of the [P, KF*DIM] w1 image
W2SLAB = KF * DIM // NSLAB   # 2304 cols of the [P, KF*DIM] w2 image


def _chunks(total):
    """512-token chunks (one PSUM bank of fp32 per fc1 matmul output)."""
    out, pos = [], 0
    while pos < total:
        w = min(512, total - pos)
        out.append((pos, w))
        pos += w
    return out


def _token_tiles(w):
    tiles, off = [], 0
    while off < w:
        p = min(P, w - off)
        tiles.append((off, p))
        off += p
    return tiles


def _body(tc, xb_d, w1_d, b1_d, w2_d, b2_d, o_d, t_tokens):
    nc = tc.nc
    with (
        tc.tile_pool(name="const", bufs=1) as constp,
        tc.tile_pool(name="wpool", bufs=1) as wp,
        tc.tile_pool(name="xt", bufs=3) as xtp,
        tc.tile_pool(name="ht", bufs=50) as htp,
        tc.tile_pool(name="onat", bufs=4) as onatp,
        tc.tile_pool(name="ps1", bufs=4, space=bass.MemorySpace.PSUM) as ps1p,
        tc.tile_pool(name="ps2", bufs=2, space=bass.MemorySpace.PSUM) as ps2p,
    ):
        b1_s = constp.tile([P, KF], F32)
        nc.scalar.dma_start(out=b1_s[:], in_=b1_d)
        b2_s = constp.tile([P, DIM], F32)
        nc.scalar.dma_start(out=b2_s[:], in_=b2_d)

        # Warm-ups during the weight-DMA wait: absorb the ~1.3us Gelu table
        # load on ACT, and keep the PE busy until real matmuls arrive so the
        # HAM clock-gate sits at 8/8 (else the first ~3.4us run at half rate).
        # memset (not DMA) sources the operands so nothing blocks them.
        warm = constp.tile([P, 512], BF16)
        nc.vector.memset(warm[:, :], 0.0)
        warm2 = constp.tile([P, 8], BF16)
        nc.scalar.activation(warm2[:, :], warm[:, 0:8], _ACT_FUNC, bias=0.0)
        psw = ps1p.tile([P, 512], F32, tag="ps1")
        for _ in range(26):
            nc.tensor.matmul(psw[:, :], warm[:, 0:P], warm[:, :],
                             start=True, stop=True)

        # Weight SBUF images, streamed in 8 contiguous slabs each, spread
        # over two DMA queues. w1 image is m-major: col m*768 + k*128 + c
        # holds W1mT[128k+c_row...]; fc1's m-chain for m only needs slab
        # m//3, so matmuls start after ~590KB instead of all of W1.
        chunks = _chunks(t_tokens)

        def load_x(c0, cw):
            """One contiguous DMA for a whole pre-blocked token chunk."""
            xt = xtp.tile([P, KD * cw], BF16, tag="xt", name="xt")
            nc.gpsimd.dma_start(
                out=xt[:, :], in_=xb_d[:, KD * c0:KD * (c0 + cw)]
            )
            return xt

        xt_cur = load_x(*chunks[0])
        # prefetch chunk 1 too: its fc1 interleaves with chunk 0's below
        xt_nxt = load_x(*chunks[1]) if len(chunks) > 1 else None

        w1_s = wp.tile([P, KF * DIM], BF16, tag="w1", name="w1")
        w2_s = wp.tile([P, KF * DIM], BF16, tag="w2", name="w2")
        NS = 6
        SW = KF * DIM // NS           # 3072 cols -> 6KB descriptors,
        # first slab split in half so fc1's m=0 chain starts ~4us sooner
        w1_bounds = [0, SW // 2, SW] + [(s + 1) * SW for s in range(1, NS)]
        w1_q = [nc.sync, nc.scalar, nc.gpsimd, nc.sync, nc.scalar,
                nc.gpsimd, nc.sync]
        for i, (a, b) in enumerate(zip(w1_bounds, w1_bounds[1:])):
            w1_q[i].dma_start(out=w1_s[:, a:b], in_=w1_d[:, a:b])
        for s in range(NS):
            q = nc.sync if s % 2 == 1 else nc.scalar
            q.dma_start(
                out=w2_s[:, s * SW:(s + 1) * SW],
                in_=w2_d[:, s * SW:(s + 1) * SW],
            )

        out_q = [nc.sync, nc.scalar]
        out_n = 0

        def fc1_chain(xt, cw, m):
            ps1 = ps1p.tile([P, cw], F32, tag="ps1")
            for k in range(KD):
                nc.tensor.matmul(
                    ps1[:, :],
                    w1_s[:, m * DIM + k * P:m * DIM + (k + 1) * P],
                    xt[:, k * cw:(k + 1) * cw],
                    start=(k == 0), stop=(k == KD - 1),
                )
            ht = htp.tile([P, cw], BF16, tag="ht")
            nc.scalar.activation(
                ht[:, :], ps1[:, :], _ACT_FUNC, bias=b1_s[:, m:m + 1]
            )
            return ht

        def fc2_chunk(c0, cw, hts):
            nonlocal out_n
            for (toff, tp) in _token_tiles(cw):
                ps2 = ps2p.tile([P, DIM], F32, tag="ps2")
                for k in range(KF):
                    last = (k == KF - 1)
                    for off, wdt in ((0, 512), (512, DIM - 512)):
                        nc.tensor.matmul(
                            ps2[0:tp, off:off + wdt],
                            hts[k][:, toff:toff + tp],
                            w2_s[:, k * DIM + off:k * DIM + off + wdt],
                            start=(k == 0), stop=last,
                        )
                on = onatp.tile([P, DIM], BF16, tag="on")
                nc.vector.tensor_tensor(
                    out=on[0:tp, :], in0=ps2[0:tp, :], in1=b2_s[0:tp, :],
                    op=mybir.AluOpType.add,
                )
                # split the store across two DMA queues so the last store
                # isn't a single ~9us single-engine transfer
                for off, wdt in ((0, 384), (384, 384)):
                    out_q[out_n % 2].dma_start(
                        out=o_d[c0 + toff:c0 + toff + tp, off:off + wdt],
                        in_=on[0:tp, off:off + wdt],
                    )
                    out_n += 1

        # Prologue: interleave chunk 0 and chunk 1 fc1 m-chains so the PE
        # has 2x the compute per W1 slab while the cold DMA rings are still
        # delivering weights (else fc1 outruns the slab stream and stalls).
        if xt_nxt is not None:
            hts0, hts1 = [], []
            for m in range(KF):
                hts0.append(fc1_chain(xt_cur, chunks[0][1], m))
                hts1.append(fc1_chain(xt_nxt, chunks[1][1], m))
            fc2_chunk(chunks[0][0], chunks[0][1], hts0)
            xt_cur = load_x(*chunks[2]) if len(chunks) > 2 else None
            fc2_chunk(chunks[1][0], chunks[1][1], hts1)
            start_ci = 2
        else:
            hts0 = [fc1_chain(xt_cur, chunks[0][1], m) for m in range(KF)]
            fc2_chunk(chunks[0][0], chunks[0][1], hts0)
            start_ci = 1

        for ci in range(start_ci, len(chunks)):
            c0, cw = chunks[ci]
            hts = [fc1_chain(xt_cur, cw, m) for m in range(KF)]
            xt_next = (load_x(*chunks[ci + 1])
                       if ci + 1 < len(chunks) else None)
            fc2_chunk(c0, cw, hts)
            xt_cur = xt_next


def build_program(t_tokens=T):
    nc = bacc.Bacc("TRN2", target_bir_lowering=False, debug=False,
                   num_devices=NCORES)
    xb_d = nc.dram_tensor("xb", [P, KD * t_tokens], BF16,
                          kind="ExternalInput").ap()
    w1_d = nc.dram_tensor("w1b", [P, KF * DIM], BF16,
                          kind="ExternalInput").ap()
    b1_d = nc.dram_tensor("b1", [P, KF], F32, kind="ExternalInput").ap()
    w2_d = nc.dram_tensor("w2b", [P, KF * DIM], BF16,
                          kind="ExternalInput").ap()
    b2_d = nc.dram_tensor("b2", [P, DIM], F32, kind="ExternalInput").ap()
    o_d = nc.dram_tensor("out", [t_tokens, DIM], BF16,
                         kind="ExternalOutput").ap()
    with tile.TileContext(nc) as tc:
        _body(tc, xb_d, w1_d, b1_d, w2_d, b2_d, o_d, t_tokens)
    nc.compile()
    return nc


def host_prep(x, W1, b1, W2, b2, mask1, mask2):
    x2 = np.asarray(x, dtype=np.float32).reshape(TOK, DIM)
    m1 = np.repeat(np.repeat(np.asarray(mask1, dtype=bool), BLK, 0), BLK, 1)
    m2 = np.repeat(np.repeat(np.asarray(mask2, dtype=bool), BLK, 0), BLK, 1)
    w1t = (np.asarray(W1, np.float32) * m1.astype(np.float32)).T  # [DIM, FF]
    w2t = (np.asarray(W2, np.float32) * m2.astype(np.float32)).T  # [FF, DIM]
    # m-major blocked image: w1b[p, m*768 + k*128 + c] = w1t[k*128+p, m*128+c]
    w1b = np.ascontiguousarray(
        w1t.reshape(KD, P, KF, P).transpose(1, 2, 0, 3).reshape(P, KF * DIM)
    ).astype(BF)
    # w2b[p, m*768 + d] = w2t[m*128+p, d]
    w2b = np.ascontiguousarray(
        w2t.reshape(KF, P, DIM).transpose(1, 0, 2).reshape(P, KF * DIM)
    ).astype(BF)
    b1h = np.ascontiguousarray(
        np.asarray(b1, np.float32).reshape(KF, P).T)              # [P, KF]
    b2h = np.ascontiguousarray(
        np.broadcast_to(np.asarray(b2, np.float32)[None, :], (P, DIM)))
    chunks = _chunks(T)
    xbs = []
    for c in range(NCORES):
        xt_c = x2[c * T:(c + 1) * T].T                            # [DIM, T]
        xb_c = np.concatenate(
            [xt_c[:, c0:c0 + cw].reshape(KD, P, cw).transpose(1, 0, 2)
             .reshape(P, KD * cw) for (c0, cw) in chunks], axis=1)
        xbs.append(np.ascontiguousarray(xb_c).astype(BF))
    return xbs, w1b, b1h, w2b, b2h


_PROGRAM = None


def _get_program():
    global _PROGRAM
    if _PROGRAM is None:
        _PROGRAM = build_program(T)
    return _PROGRAM


def kernel(x, W1, b1, W2, b2, mask1, mask2, **run_kwargs):
    xbs, w1b, b1h, w2b, b2h = host_prep(x, W1, b1, W2, b2, mask1, mask2)
    nc = _get_program()
    in_maps = [
        {"xb": xbs[c], "w1b": w1b, "b1": b1h, "w2b": w2b, "b2": b2h}
        for c in range(NCORES)
    ]
    res = run_bass_kernel_spmd(nc, in_maps, list(range(NCORES)), **run_kwargs)
    out = np.concatenate([res.results[c]["out"] for c in range(NCORES)], axis=0)
    out = out.astype(np.float32).reshape(B, S, DIM)
    if run_kwargs:
        kernel.last_results = res
    return out
